# revision 8
# baseline (speedup 1.0000x reference)
"""DualAttention2d Trainium2 kernel, pair-split design.

Sharding: 8 cores = 4 samples x 2 row-halves. Cores (2b, 2b+1) split sample b
by image rows (0-31 / 32-63). Every core runs the SAME instruction stream
(no tc.If): it computes BOTH branches' convs on its own row-half, its half of
the spatial attention (split over query positions k == its rows), and its
half of the channel attention. Branch outputs are summed on-core; the host
just concatenates row-halves.

Uniformity trick: the odd core's half is stored VERTICALLY MIRRORED (host
flips x rows, conv taps dy<->2-dy, and un-flips the output), so "my boundary
row" / "halo row" land at the same local pad rows (32 / 33) on both cores.

Cross-core (pair) data movement, all hidden behind compute:
 - AllGather of keys (fp32) and v^T (bf16) after sa_conv1,
 - AllReduce of the Gram matrix G after ca_conv1,
 - AllReduce halo exchange (partner = sum - own) of the two post-middle
   boundary rows before conv2.

Spatial softmax with no big transposes: pass1 computes logits in [k, s]
layout only to get row maxes (PSUM discarded); the max is folded into a
65th contraction row (khat gets ones, qhat gets -m), so pass2 produces
exp-ready TRANSPOSED logits [s, k] directly; exp goes PSUM->bf16 SBUF; the
o-matmul consumes expT as stationary (plus an N=1 ones-matmul for the row
sums, which land per-partition exactly where the reciprocal scale needs
them); only the final [k, c] -> [c, k] flip uses PE transposes (64 per core
vs 1024 in the per-branch design).

Conv weights are host-packed in SBUF layout (one contiguous DMA per output
block) and loaded once per conv (ob-outer loop), not once per s-pair.
conv2 runs in bf16 (weights + inputs); conv1 stays fp32r because its output
feeds the attention logits.
"""

import numpy as np

import concourse.bacc as bacc
import concourse.mybir as mybir
import concourse.tile as tile
from concourse.bass_utils import run_bass_kernel_spmd

B, C, H, W = 4, 512, 64, 64
S = H * W            # 4096
CI = 64              # q/k channels
P = 128
NB = C // P          # 4 channel blocks
RH = 32              # own rows per core
SH = RH * W          # 2048 own spatial positions
NSTH = SH // 512     # 4 local s-tiles
NCHH = SH // P       # 16 local s-chunks
PW = 66              # padded row width
PADR = RH + 2        # padded rows (pad row 0, own rows 1..32, halo row 33)
PAD = PW * PADR      # 2244
NKG = 8              # k groups
KG = SH // NKG       # 256 query positions per group
EPS = 1e-5
GROUPS = [[0, 1], [2, 3], [4, 5], [6, 7]]

F32 = mybir.dt.float32
F32R = mybir.dt.float32r
BF16 = mybir.dt.bfloat16
AF = mybir.ActivationFunctionType
AX = mybir.AxisListType

_CACHE = {}


def build(reps=1, debug=False):
    nc = bacc.Bacc("TRN2", target_bir_lowering=False, debug=False,
                   num_devices=8)

    # ---- I/O ----
    x_d = nc.dram_tensor("xpad", [NB, P, PAD], F32R, kind="ExternalInput")
    w1s_d = nc.dram_tensor("w1s", [NB, P, 36 * P], F32R, kind="ExternalInput")
    w1c_d = nc.dram_tensor("w1c", [NB, P, 36 * P], F32R, kind="ExternalInput")
    w2s_d = nc.dram_tensor("w2s", [NB, P, 36 * P], BF16, kind="ExternalInput")
    w2c_d = nc.dram_tensor("w2c", [NB, P, 36 * P], BF16, kind="ExternalInput")
    b1s_d = nc.dram_tensor("b1s", [NB, P, 1], F32, kind="ExternalInput")
    b1c_d = nc.dram_tensor("b1c", [NB, P, 1], F32, kind="ExternalInput")
    b2s_d = nc.dram_tensor("b2s", [NB, P, 1], F32, kind="ExternalInput")
    b2c_d = nc.dram_tensor("b2c", [NB, P, 1], F32, kind="ExternalInput")
    qw_d = nc.dram_tensor("qw", [NB, P, CI], F32R, kind="ExternalInput")
    kw_d = nc.dram_tensor("kw", [NB, P, CI], F32R, kind="ExternalInput")
    vw_d = nc.dram_tensor("vw", [NB, P, 512], F32R, kind="ExternalInput")
    qb_d = nc.dram_tensor("qb", [CI, 1], F32, kind="ExternalInput")
    kb_d = nc.dram_tensor("kb", [CI, 1], F32, kind="ExternalInput")
    vba_d = nc.dram_tensor("vba", [NB, P, 1], F32, kind="ExternalInput")
    beta_d = nc.dram_tensor("betat", [P, 1], F32, kind="ExternalInput")
    idr_d = nc.dram_tensor("identr", [P, P], F32R, kind="ExternalInput")
    ones_d = nc.dram_tensor("onesrow", [1, S], F32R, kind="ExternalInput")
    onescol_d = nc.dram_tensor("onescol", [P, 1], BF16, kind="ExternalInput")
    zeros_d = nc.dram_tensor("zerospad", [P, PAD], BF16, kind="ExternalInput")
    out_d = nc.dram_tensor("out", [NB, P, SH], F32, kind="ExternalOutput")
    dbg_d = (nc.dram_tensor("dbg", [26, P, S], F32, kind="ExternalOutput")
             if debug else None)

    # ---- internal DRAM ----
    kq_in = nc.dram_tensor("kqin", [CI, SH], F32R, kind="Internal")
    kq_out = nc.dram_tensor("kqout", [2, CI, SH], F32R, kind="Internal")
    vt_in = nc.dram_tensor("vtin", [NCHH, P, 512], BF16, kind="Internal")
    vt_out = nc.dram_tensor("vtout", [2, NCHH, P, 512], BF16, kind="Internal")
    c1t_dd = nc.dram_tensor("c1t", [NCHH, P, 512], F32R, kind="Internal")
    g_in = nc.dram_tensor("gin", [NB, P, 512], F32, kind="Internal")
    g_out = nc.dram_tensor("gout", [NB, P, 512], F32, kind="Internal")
    halo_in = nc.dram_tensor("haloin", [2, NB, P, W], F32, kind="Internal")
    halo_out = nc.dram_tensor("haloout", [2, NB, P, W], F32, kind="Internal")
    mrow_d = nc.dram_tensor("mrow", [NKG, KG], F32R, kind="Internal")

    def pv(t, st, dy=1, dx=1):
        """[128, 8, 64] view of a padded tile covering local s-tile st,
        shifted by conv tap (dy, dx)."""
        v = t.rearrange("p (r w) -> p r w", w=PW)
        r0 = st * 8 + dy
        return v[:, r0:r0 + 8, dx:dx + W]

    def cv(t, ch):
        """[128, 2, 64] centered view covering local s-chunk ch."""
        v = t.rearrange("p (r w) -> p r w", w=PW)
        r0 = ch * 2 + 1
        return v[:, r0:r0 + 2, 1:1 + W]

    def rowv(t, r):
        """[128, 64] centered view of local pad row r."""
        v = t.rearrange("p (r w) -> p r w", w=PW)
        return v[:, r, 1:1 + W]

    def gv(t, g):
        """[128, 4, 64] centered view covering k-group g (4 image rows)."""
        v = t.rearrange("p (r w) -> p r w", w=PW)
        r0 = g * 4 + 1
        return v[:, r0:r0 + 4, 1:1 + W]

    with tile.TileContext(nc) as tc:
        from contextlib import ExitStack

        gctx = ExitStack()
        psA = gctx.enter_context(tc.tile_pool(name="psA", bufs=6,
                                              space="PSUM"))
        psT = gctx.enter_context(tc.tile_pool(name="psT", bufs=2,
                                              space="PSUM"))
        consts = gctx.enter_context(tc.tile_pool(name="consts", bufs=1))
        s1p = gctx.enter_context(tc.tile_pool(name="s1p", bufs=NB))
        c1p = gctx.enter_context(tc.tile_pool(name="c1p", bufs=NB))
        khatp = gctx.enter_context(tc.tile_pool(name="khatp", bufs=1))
        qhatp = gctx.enter_context(tc.tile_pool(name="qhatp", bufs=1))
        b512 = gctx.enter_context(tc.tile_pool(name="b512", bufs=3))
        statp = gctx.enter_context(tc.tile_pool(name="statp", bufs=16))

        # ---- constants ----
        idr_t = consts.tile([P, P], F32R, name="idr")
        nc.sync.dma_start(idr_t[:], idr_d.ap())
        qw_t = [consts.tile([P, CI], F32R, name=f"qw{i}") for i in range(NB)]
        kw_t = [consts.tile([P, CI], F32R, name=f"kw{i}") for i in range(NB)]
        vw_t = [consts.tile([P, 512], F32R, name=f"vw{i}") for i in range(NB)]
        b1s_t = [consts.tile([P, 1], F32, name=f"b1s{i}") for i in range(NB)]
        b1c_t = [consts.tile([P, 1], F32, name=f"b1c{i}") for i in range(NB)]
        b2s_t = [consts.tile([P, 1], F32, name=f"b2s{i}") for i in range(NB)]
        b2c_t = [consts.tile([P, 1], F32, name=f"b2c{i}") for i in range(NB)]
        vba_t = [consts.tile([P, 1], F32, name=f"vba{i}") for i in range(NB)]
        qb_t = consts.tile([CI, 1], F32, name="qb")
        kb_t = consts.tile([CI, 1], F32, name="kb")
        beta_t = consts.tile([P, 1], F32, name="beta")
        onesb_t = consts.tile([P, 1], BF16, name="onesb")
        nc.sync.dma_start(onesb_t[:], onescol_d.ap())
        for i in range(NB):
            nc.sync.dma_start(qw_t[i][:], qw_d[i])
            nc.sync.dma_start(kw_t[i][:], kw_d[i])
            nc.sync.dma_start(vw_t[i][:], vw_d[i])
            nc.sync.dma_start(b1s_t[i][:], b1s_d[i])
            nc.sync.dma_start(b1c_t[i][:], b1c_d[i])
            nc.sync.dma_start(b2s_t[i][:], b2s_d[i])
            nc.sync.dma_start(b2c_t[i][:], b2c_d[i])
            nc.sync.dma_start(vba_t[i][:], vba_d[i])
        nc.sync.dma_start(qb_t[:], qb_d.ap())
        nc.sync.dma_start(kb_t[:], kb_d.ap())
        nc.sync.dma_start(beta_t[:], beta_d.ap())

        # ---- persistent feature-map pads ----
        s1sb = [s1p.tile([P, SH], F32R, tag="s1", name=f"s1sb{i}")
                for i in range(NB)]
        c1sb = [c1p.tile([P, SH], F32R, tag="c1", name=f"c1sb{i}")
                for i in range(NB)]
        khat = khatp.tile([CI + 1, S], F32R, tag="kh", name="khat")
        qhat = qhatp.tile([CI + 1, SH], F32R, tag="qh", name="qhat")


        def conv(wres, src, ob, st, bias, dst, func=AF.Relu, bpool=None):
            """One 3x3 conv s-tile for output block ob: 36 accumulating
            matmuls, relu+bias evict into dst's centered view. With bpool,
            also evicts a contiguous copy (for PE transposes, which cannot
            read multi-free-dim strided views)."""
            ps = psA.tile([P, 512], F32, tag="mm", name="cps")
            for tci in range(36):
                cb, tap = divmod(tci, 9)
                dy, dx = divmod(tap, 3)
                nc.tensor.matmul(ps[:], wres[:, tci * P:(tci + 1) * P],
                                 pv(src[cb], st, dy, dx),
                                 start=(tci == 0), stop=(tci == 35))
            nc.scalar.activation(dst, ps[:], func, bias=bias)
            if bpool is not None:
                sb = bpool.tile([P, 512], F32R, tag="cb", name="csb")
                nc.scalar.activation(sb[:], ps[:], func, bias=bias)
                return sb
            return None

        def body(rep):
            with ExitStack() as convctx:
                xpadp = convctx.enter_context(
                    tc.tile_pool(name="xpadp", bufs=NB))
                xpad = [xpadp.tile([P, PAD], F32R, tag="xp",
                                   name=f"xpad{i}") for i in range(NB)]
                for i in range(NB):
                    hh = PAD // 2
                    nc.sync.dma_start(xpad[i][:, :hh], x_d[i, :, :hh])
                    nc.scalar.dma_start(xpad[i][:, hh:], x_d[i, :, hh:])

                # ---- P1: sa_conv1 (ob-outer), then q/k/vT projections ----
                with ExitStack() as p1:
                    wp = p1.enter_context(tc.tile_pool(name="wp1", bufs=2))
                    vtbp = p1.enter_context(tc.tile_pool(name="vtbp", bufs=3))
                    for ob in range(NB):
                        wres = wp.tile([P, 36 * P], F32R, tag="w", name="wres")
                        nc.sync.dma_start(wres[:], w1s_d[ob])
                        for st in range(NSTH):
                            conv(wres, xpad, ob, st, b1s_t[ob][:],
                                 s1sb[ob][:, st * 512:(st + 1) * 512])
                    for st in range(NSTH):
                        ssl = slice(st * 512, (st + 1) * 512)
                        pq = psA.tile([CI, 512], F32, tag="mm", name="pq")
                        for cb in range(NB):
                            nc.tensor.matmul(pq[:], qw_t[cb][:],
                                             s1sb[cb][:, ssl],
                                             start=(cb == 0),
                                             stop=(cb == NB - 1))
                        nc.scalar.activation(qhat[0:CI, ssl], pq[:],
                                             AF.Identity, bias=qb_t[:])
                        pk = psA.tile([CI, 512], F32, tag="mm", name="pk")
                        for cb in range(NB):
                            nc.tensor.matmul(pk[:], kw_t[cb][:],
                                             s1sb[cb][:, ssl],
                                             start=(cb == 0),
                                             stop=(cb == NB - 1))
                        ksb = b512.tile([CI, 512], F32R, tag="bn", name="ksb")
                        nc.scalar.activation(ksb[:], pk[:], AF.Identity,
                                             bias=kb_t[:])
                        nc.scalar.dma_start(kq_in.ap()[:, ssl], ksb[:])
                        for j in range(4):
                            ch = st * 4 + j
                            pvp = psA.tile([P, 512], F32, tag="mm", name="pv")
                            for cb in range(NB):
                                nc.tensor.matmul(
                                    pvp[:],
                                    s1sb[cb][:, ch * P:(ch + 1) * P],
                                    vw_t[cb][:], start=(cb == 0),
                                    stop=(cb == NB - 1))
                            vtb = vtbp.tile([P, 512], BF16, tag="vtb",
                                            name="vtb")
                            nc.scalar.activation(vtb[:], pvp[:], AF.Identity)
                            nc.scalar.dma_start(vt_in[ch], vtb[:])

                # CC1: pair AllGather of keys and vT
                nc.gpsimd.collective_compute(
                    "AllGather", mybir.AluOpType.bypass,
                    replica_groups=GROUPS,
                    ins=[kq_in.ap()], outs=[kq_out.ap()])
                nc.gpsimd.collective_compute(
                    "AllGather", mybir.AluOpType.bypass,
                    replica_groups=GROUPS,
                    ins=[vt_in.ap()], outs=[vt_out.ap()])
                for r in range(2):
                    nc.sync.dma_start(khat[0:CI, r * SH:(r + 1) * SH],
                                      kq_out[r])
                nc.sync.dma_start(khat[CI:CI + 1, :], ones_d.ap())

                # ---- P2: ca_conv1 + c1T + partial G ----
                with ExitStack() as p2:
                    wp = p2.enter_context(tc.tile_pool(name="wp2", bufs=2))
                    tbp = p2.enter_context(tc.tile_pool(name="tbp", bufs=3))
                    for ob in range(NB):
                        wres = wp.tile([P, 36 * P], F32R, tag="w", name="wres")
                        nc.sync.dma_start(wres[:], w1c_d[ob])
                        for st in range(NSTH):
                            csb = conv(wres, xpad, ob, st, b1c_t[ob][:],
                                       c1sb[ob][:, st * 512:(st + 1) * 512])
                            csb = c1sb[ob][:, st * 512:(st + 1) * 512]
                            tb = tbp.tile([P, 512], F32R, tag="tb", name="tb")
                            for j in range(4):
                                pt = psT.tile([P, P], F32R, tag="tp",
                                              name="pt")
                                nc.tensor.transpose(
                                    pt[:], csb[:, j * P:(j + 1) * P],
                                    idr_t[:])
                                nc.scalar.activation(tb[:, j * P:(j + 1) * P],
                                                     pt[:], AF.Identity)
                            nc.scalar.dma_start(
                                c1t_dd.ap()[st * 4:st * 4 + 4, :,
                                            ob * P:(ob + 1) * P]
                                .rearrange("j p c -> p j c"),
                                tb[:].rearrange("p (j c) -> p j c", c=P))
                    pg = [psA.tile([P, 512], F32, tag="mm", name=f"pg{cb}")
                          for cb in range(NB)]
                    with tc.tile_pool(name="c1tin", bufs=3) as cp:
                        for ch in range(NCHH):
                            ct = cp.tile([P, 512], F32R, tag="ct", name="ct")
                            nc.sync.dma_start(ct[:], c1t_dd[ch])
                            for cb in range(NB):
                                nc.tensor.matmul(
                                    pg[cb][:],
                                    ct[:, cb * P:(cb + 1) * P], ct[:],
                                    start=(ch == 0), stop=(ch == NCHH - 1))
                    for cb in range(NB):
                        gsb = b512.tile([P, 512], F32, tag="bn", name="gsb")
                        nc.scalar.activation(gsb[:], pg[cb][:], AF.Identity)
                        nc.scalar.dma_start(g_in[cb], gsb[:])

                # CC2: pair AllReduce of G
                nc.gpsimd.collective_compute(
                    "AllReduce", mybir.AluOpType.add,
                    replica_groups=GROUPS,
                    ins=[g_in.ap()], outs=[g_out.ap()])

            if debug:
                for i in range(NB):
                    nc.gpsimd.dma_start(dbg_d[i, :, :SH], s1sb[i][:])
                    nc.gpsimd.dma_start(dbg_d[4 + i, :, :SH], c1sb[i][:])
                nc.gpsimd.dma_start(dbg_d[8, 0:CI + 1, :], khat[:])

            # xpad freed; open result pads + attention pools
            with ExitStack() as rctx2:
                srp = rctx2.enter_context(tc.tile_pool(name="srp", bufs=NB))
                crp = rctx2.enter_context(tc.tile_pool(name="crp", bufs=NB))
                sres = [srp.tile([P, PAD], BF16, tag="sr", name=f"sres{i}")
                        for i in range(NB)]
                cres = [crp.tile([P, PAD], BF16, tag="cr", name=f"cres{i}")
                        for i in range(NB)]
                for i in range(NB):
                    nc.sync.dma_start(sres[i][:], zeros_d.ap())
                    nc.scalar.dma_start(cres[i][:], zeros_d.ap())

                # ---- P3: spatial attention, pipelined over 8 k-groups ----
                with ExitStack() as p3:
                    vtfp = p3.enter_context(tc.tile_pool(name="vtfp",
                                                         bufs=1))
                    vtfull = vtfp.tile([P, 32 * 512], BF16, tag="vtf",
                                       name="vtfull")
                    for r in range(2):
                        nc.sync.dma_start(
                            vtfull[:, r * 16 * 512:(r + 1) * 16 * 512]
                            .rearrange("p (j n) -> p j n", n=512),
                            vt_out[r].rearrange("j p n -> p j n"))
                    expTp = p3.enter_context(tc.tile_pool(name="expTp",
                                                          bufs=2))
                    eTs, rsts = {}, {}

                    def issue_L(g):
                        nm = statp.tile([P, 2], F32R, tag="st", name="nm")
                        for kci in range(2):
                            kc = g * 2 + kci
                            pmax = statp.tile([P, 8], F32, tag="st",
                                              name="pmax")
                            for st8 in range(8):
                                pl = psA.tile([P, 512], F32, tag="mm",
                                              name="pl")
                                nc.tensor.matmul(
                                    pl[:], qhat[0:CI, kc * P:(kc + 1) * P],
                                    khat[0:CI, st8 * 512:(st8 + 1) * 512],
                                    start=True, stop=True)
                                nc.vector.reduce_max(pmax[:, st8:st8 + 1],
                                                     pl[:], axis=AX.X)
                            nc.vector.reduce_max(nm[:, kci:kci + 1], pmax[:],
                                                 axis=AX.X, negate=True)
                        nc.scalar.dma_start(
                            mrow_d.ap()[g].rearrange("(j p) -> p j", p=P),
                            nm[:])
                        nc.sync.dma_start(
                            qhat[CI:CI + 1, g * KG:(g + 1) * KG],
                            mrow_d.ap()[g:g + 1, :])

                    def issue_M(g):
                        eT = expTp.tile([P, 32 * KG], BF16, tag="eT",
                                        name="eT")
                        for sc in range(32):
                            lt = psA.tile([P, KG], F32, tag="mm", name="lt")
                            nc.tensor.matmul(
                                lt[:], khat[0:CI + 1, sc * P:(sc + 1) * P],
                                qhat[0:CI + 1, g * KG:(g + 1) * KG],
                                start=True, stop=True)
                            nc.scalar.activation(eT[:, sc * KG:(sc + 1) * KG],
                                                 lt[:], AF.Exp)
                        eTs[g] = eT

                    def issue_O(g):
                        eT = eTs.pop(g)
                        if debug and g == 0:
                            nc.gpsimd.dma_start(dbg_d[9, 0:CI + 1, :SH],
                                                qhat[:])
                            nc.gpsimd.dma_start(
                                dbg_d[10, :, :],
                                eT[:, :S])
                            nc.gpsimd.dma_start(
                                dbg_d[11, :, :],
                                eT[:, S:2 * S])
                        po = [psA.tile([P, 512], F32, tag="mm",
                                       name=f"po{i}") for i in range(2)]
                        # start=True clears the WHOLE PSUM bank, so each
                        # rowsum accumulator needs its own bank
                        prs = [psA.tile([P, 1], F32, tag="mm",
                                        name=f"prs{i}") for i in range(2)]
                        for sc in range(32):
                            vt = vtfull[:, sc * 512:(sc + 1) * 512]
                            for kci in range(2):
                                esl = eT[:, sc * KG + kci * P:
                                         sc * KG + (kci + 1) * P]
                                nc.tensor.matmul(po[kci][:], esl, vt,
                                                 start=(sc == 0),
                                                 stop=(sc == 31))
                                nc.tensor.matmul(
                                    prs[kci][:], esl,
                                    onesb_t[:], start=(sc == 0),
                                    stop=(sc == 31))
                        rsts[g] = (po, prs)

                    def issue_R(g):
                        po, prs = rsts.pop(g)
                        rs = statp.tile([P, 2], F32, tag="st", name="rs")
                        for kci in range(2):
                            nc.vector.reciprocal(rs[:, kci:kci + 1],
                                                 prs[kci][:])
                        if debug:
                            prsb = statp.tile([P, 2], F32, tag="st",
                                              name="prsb")
                            for kci in range(2):
                                nc.scalar.activation(prsb[:, kci:kci + 1],
                                                     prs[kci][:], AF.Identity)
                            nc.gpsimd.dma_start(
                                dbg_d[24, :, 4 * g:4 * g + 2], prsb[:])
                            nc.gpsimd.dma_start(
                                dbg_d[24, :, 4 * g + 2:4 * g + 4], rs[:])
                            pob = b512.tile([P, 512], F32, tag="bn",
                                            name="pob")
                            nc.scalar.activation(pob[:], po[0][:],
                                                 AF.Identity)
                            nc.gpsimd.dma_start(
                                dbg_d[25, :, 512 * (g % 8):512 * (g % 8 + 1)],
                                pob[:])
                        posb = []
                        for kci in range(2):
                            t = b512.tile([P, 512], F32R, tag="bn",
                                          name="posb")
                            nc.scalar.activation(t[:], po[kci][:],
                                                 AF.Identity,
                                                 scale=rs[:, kci:kci + 1])
                            posb.append(t)
                        for cb in range(NB):
                            ptp = psT.tile([P, KG], F32R, tag="tp",
                                           name="ptp")
                            for kci in range(2):
                                nc.tensor.transpose(
                                    ptp[:, kci * P:(kci + 1) * P],
                                    posb[kci][:, cb * P:(cb + 1) * P],
                                    idr_t[:])
                            ob_sb = b512.tile([P, KG], F32, tag="bn",
                                              name="obsb")
                            nc.scalar.activation(ob_sb[:], ptp[:],
                                                 AF.Identity,
                                                 bias=vba_t[cb][:])
                            nc.vector.tensor_add(
                                gv(sres[cb], g),
                                ob_sb[:].rearrange("p (r w) -> p r w", w=W),
                                s1sb[cb][:, g * KG:(g + 1) * KG]
                                .rearrange("p (r w) -> p r w", w=W))

                    issue_L(0)
                    issue_L(1)
                    issue_M(0)
                    for g in range(NKG):
                        if g + 2 < NKG:
                            issue_L(g + 2)
                        if g + 1 < NKG:
                            issue_M(g + 1)
                        issue_O(g)
                        issue_R(g)

                # ---- P4: channel middle ----
                with tc.tile_pool(name="cattp", bufs=NB) as cattp:
                    catt = []
                    for cb in range(NB):
                        gsb = b512.tile([P, 512], F32, tag="bn", name="gsb2")
                        nc.sync.dma_start(gsb[:], g_out[cb])
                        negmax = statp.tile([P, 1], F32, tag="st",
                                            name="negmax")
                        nc.vector.reduce_max(negmax[:], gsb[:], axis=AX.X,
                                             negate=True)
                        ct = cattp.tile([P, 512], F32R, tag="ct",
                                        name=f"catt{cb}")
                        rowsum = statp.tile([P, 1], F32, tag="st",
                                            name="rowsum")
                        nc.scalar.activation(ct[:], gsb[:], AF.Exp,
                                             bias=negmax[:],
                                             accum_out=rowsum[:])
                        recip = statp.tile([P, 1], F32, tag="st",
                                           name="recip")
                        nc.vector.reciprocal(recip[:], rowsum[:])
                        nc.vector.tensor_mul(recip[:], recip[:], beta_t[:])
                        nc.scalar.activation(ct[:], ct[:], AF.Identity,
                                             scale=recip[:])
                        catt.append(ct)
                    for st in range(NSTH):
                        pc4 = [psA.tile([P, 512], F32, tag="mm",
                                        name=f"pc{kb}") for kb in range(NB)]
                        for kb in range(NB):
                            for cb in range(NB):
                                nc.tensor.matmul(
                                    pc4[kb][:],
                                    catt[cb][:, kb * P:(kb + 1) * P],
                                    c1sb[cb][:, st * 512:(st + 1) * 512],
                                    start=(cb == 0),
                                    stop=(cb == NB - 1))
                        for kb in range(NB):
                            nc.vector.tensor_add(
                                pv(cres[kb], st), pc4[kb][:],
                                c1sb[kb][:, st * 512:(st + 1) * 512]
                                .rearrange("p (r w) -> p r w", w=W))

                # ---- CC3: halo rows (partner = sum - own) ----
                with tc.tile_pool(name="halop", bufs=1) as hp:
                    hsb = hp.tile([P, 2 * NB * W], F32, name="hsb")
                    for t, buf in ((0, sres), (1, cres)):
                        for b in range(NB):
                            nc.vector.tensor_copy(
                                hsb[:, (t * NB + b) * W:(t * NB + b + 1) * W],
                                rowv(buf[b], RH))
                    for t in range(2):
                        for b in range(NB):
                            nc.scalar.dma_start(
                                halo_in[t, b],
                                hsb[:, (t * NB + b) * W:(t * NB + b + 1) * W])
                    nc.gpsimd.collective_compute(
                        "AllReduce", mybir.AluOpType.add,
                        replica_groups=GROUPS,
                        ins=[halo_in.ap()], outs=[halo_out.ap()])
                    hob = hp.tile([P, 2 * NB * W], F32, name="hob")
                    for t in range(2):
                        for b in range(NB):
                            nc.sync.dma_start(
                                hob[:, (t * NB + b) * W:(t * NB + b + 1) * W],
                                halo_out[t, b])
                    hneg = hp.tile([P, 2 * NB * W], F32, name="hneg")
                    nc.vector.tensor_scalar_mul(hneg[:], hsb[:], -1.0)
                    nc.vector.tensor_add(hob[:], hob[:], hneg[:])
                    for t, buf in ((0, sres), (1, cres)):
                        for b in range(NB):
                            nc.vector.tensor_copy(
                                rowv(buf[b], RH + 1),
                                hob[:, (t * NB + b) * W:(t * NB + b + 1) * W])

                if debug:
                    for i in range(NB):
                        nc.gpsimd.dma_start(dbg_d[12 + i, :, :PAD],
                                            sres[i][:])
                        nc.gpsimd.dma_start(dbg_d[16 + i, :, :PAD],
                                            cres[i][:])
                        gdb = b512.tile([P, 512], F32, tag="bn", name="gdb")
                        nc.sync.dma_start(gdb[:], g_out[i])
                        nc.gpsimd.dma_start(dbg_d[20 + i, :, :512], gdb[:])

                # ---- P5: conv2s (bf16), summed on the fly ----
                with ExitStack() as p5:
                    wp = p5.enter_context(tc.tile_pool(name="wp5", bufs=4))
                    osbp = p5.enter_context(tc.tile_pool(name="osbp", bufs=4))
                    for ob in range(NB):
                        ws = wp.tile([P, 36 * P], BF16, tag="w", name="w2s")
                        nc.sync.dma_start(ws[:], w2s_d[ob])
                        wc = wp.tile([P, 36 * P], BF16, tag="w", name="w2c")
                        nc.sync.dma_start(wc[:], w2c_d[ob])
                        for st in range(NSTH):
                            pss = psA.tile([P, 512], F32, tag="mm",
                                           name="pss")
                            psc = psA.tile([P, 512], F32, tag="mm",
                                           name="psc")
                            for tci in range(36):
                                cb, tap = divmod(tci, 9)
                                dy, dx = divmod(tap, 3)
                                nc.tensor.matmul(
                                    pss[:], ws[:, tci * P:(tci + 1) * P],
                                    pv(sres[cb], st, dy, dx),
                                    start=(tci == 0), stop=(tci == 35))
                            for tci in range(36):
                                cb, tap = divmod(tci, 9)
                                dy, dx = divmod(tap, 3)
                                nc.tensor.matmul(
                                    psc[:], wc[:, tci * P:(tci + 1) * P],
                                    pv(cres[cb], st, dy, dx),
                                    start=(tci == 0), stop=(tci == 35))
                            osb = osbp.tile([P, 512], F32, tag="o",
                                            name="osb")
                            osc = osbp.tile([P, 512], F32, tag="o",
                                            name="osc")
                            nc.scalar.activation(osb[:], pss[:], AF.Relu,
                                                 bias=b2s_t[ob][:])
                            nc.scalar.activation(osc[:], psc[:], AF.Relu,
                                                 bias=b2c_t[ob][:])
                            nc.vector.tensor_add(osb[:], osb[:], osc[:])
                            nc.sync.dma_start(
                                out_d[ob, :, st * 512:(st + 1) * 512],
                                osb[:])

        for rep in range(reps):
            body(rep)

        gctx.close()

    nc.compile()
    return nc


def _fold_conv(w, g, b, m, v, flip, bf16=False):
    scale = g / np.sqrt(v + EPS)
    wf = (np.asarray(w, np.float32) * scale[:, None, None, None])
    bf = (np.asarray(b, np.float32) - np.asarray(m, np.float32) * scale)
    if flip:
        wf = wf[:, :, ::-1, :]          # mirror dy
    # [O, CIn, 3, 3] -> [ob, ci, ((cb tap) o)]
    wt = wf.transpose(2, 3, 1, 0).reshape(9, NB, P, NB, P).transpose(
        3, 1, 0, 2, 4).reshape(NB, 36, P, P).transpose(0, 2, 1, 3).reshape(
        NB, P, 36 * P)
    if bf16:
        import ml_dtypes
        wt = wt.astype(ml_dtypes.bfloat16)
    else:
        wt = wt.astype(np.float32)
    return np.ascontiguousarray(wt), bf.astype(np.float32).reshape(NB, P, 1)


def _pad_half(x, h):
    """x [C, 64, 64] -> padded own-half [NB, P, PAD] for parity h.
    h=1 is vertically mirrored so the halo row is at local row 33 on both."""
    xr = x.reshape(NB, P, H, W)
    if h == 1:
        xr = xr[:, :, ::-1, :]
    xp = np.zeros((NB, P, PADR, PW), np.float32)
    xp[:, :, 1:PADR, 1:1 + W] = xr[:, :, 0:RH + 1]
    return np.ascontiguousarray(xp.reshape(NB, P, PAD))


def prep_inputs(inputs):
    x = np.asarray(inputs["x"], np.float32)
    alpha = float(np.asarray(inputs["alpha"]).reshape(-1)[0])
    beta = float(np.asarray(inputs["beta"]).reshape(-1)[0])

    per_parity = []
    for h in range(2):
        w1s, b1s = _fold_conv(inputs["sa_w1"], inputs["sa_g1"],
                              inputs["sa_b1"], inputs["sa_m1"],
                              inputs["sa_v1"], flip=(h == 1))
        w2s, b2s = _fold_conv(inputs["sa_w2"], inputs["sa_g2"],
                              inputs["sa_b2"], inputs["sa_m2"],
                              inputs["sa_v2"], flip=(h == 1), bf16=True)
        w1c, b1c = _fold_conv(inputs["ca_w1"], inputs["ca_g1"],
                              inputs["ca_b1"], inputs["ca_m1"],
                              inputs["ca_v1"], flip=(h == 1))
        w2c, b2c = _fold_conv(inputs["ca_w2"], inputs["ca_g2"],
                              inputs["ca_b2"], inputs["ca_m2"],
                              inputs["ca_v2"], flip=(h == 1), bf16=True)
        per_parity.append((w1s, b1s, w2s, b2s, w1c, b1c, w2c, b2c))

    qw = np.ascontiguousarray(
        np.asarray(inputs["q_w"], np.float32).T.reshape(NB, P, CI))
    kw = np.ascontiguousarray(
        np.asarray(inputs["k_w"], np.float32).T.reshape(NB, P, CI))
    vw = np.ascontiguousarray(
        (alpha * np.asarray(inputs["v_w"], np.float32)).T.reshape(NB, P, 512))
    qb = np.asarray(inputs["q_b"], np.float32).reshape(CI, 1)
    kb = np.asarray(inputs["k_b"], np.float32).reshape(CI, 1)
    vba = (alpha * np.asarray(inputs["v_b"], np.float32)).reshape(NB, P, 1)
    betat = np.full((P, 1), beta, np.float32)
    identr = np.eye(P, dtype=np.float32)
    onesrow = np.ones((1, S), np.float32)
    import ml_dtypes
    onescol = np.ones((P, 1), ml_dtypes.bfloat16)
    zerospad = np.zeros((P, PAD), ml_dtypes.bfloat16)

    maps = []
    for core in range(8):
        b, h = core // 2, core % 2
        w1s, b1s, w2s, b2s, w1c, b1c, w2c, b2c = per_parity[h]
        m = dict(xpad=_pad_half(x[b], h),
                 w1s=w1s, b1s=b1s, w2s=w2s, b2s=b2s,
                 w1c=w1c, b1c=b1c, w2c=w2c, b2c=b2c,
                 qw=qw, kw=kw, vw=vw, qb=qb, kb=kb, vba=vba, betat=betat,
                 identr=identr, onesrow=onesrow, onescol=onescol,
                 zerospad=zerospad)
        maps.append(m)
    return maps


def kernel(**inputs):
    if "nc" not in _CACHE:
        _CACHE["nc"] = build()
    nc = _CACHE["nc"]
    maps = prep_inputs(inputs)
    res = run_bass_kernel_spmd(nc, maps, core_ids=list(range(8)))
    out = np.zeros((B, C, H, W), np.float32)
    for b in range(B):
        top = res.results[2 * b]["out"].reshape(C, RH, W)
        bot = res.results[2 * b + 1]["out"].reshape(C, RH, W)[:, ::-1, :]
        out[b, :, :RH] = top
        out[b, :, RH:] = bot
    return out


# revision 10
# speedup vs baseline: 1.0205x; 1.0205x over previous
"""DualAttention2d Trainium2 kernel, pair-split design.

Sharding: 8 cores = 4 samples x 2 row-halves. Cores (2b, 2b+1) split sample b
by image rows (0-31 / 32-63). Every core runs the SAME instruction stream
(no tc.If): it computes BOTH branches' convs on its own row-half, its half of
the spatial attention (split over query positions k == its rows), and its
half of the channel attention. Branch outputs are summed on-core; the host
just concatenates row-halves.

Uniformity trick: the odd core's half is stored VERTICALLY MIRRORED (host
flips x rows, conv taps dy<->2-dy, and un-flips the output), so "my boundary
row" / "halo row" land at the same local pad rows (32 / 33) on both cores.

Cross-core (pair) data movement, all hidden behind compute:
 - AllGather of keys (fp32) and v^T (bf16) after sa_conv1,
 - AllReduce of the Gram matrix G after ca_conv1,
 - AllReduce halo exchange (partner = sum - own) of the two post-middle
   boundary rows before conv2.

Spatial softmax with no big transposes: pass1 computes logits in [k, s]
layout only to get row maxes (PSUM discarded); the max is folded into a
65th contraction row (khat gets ones, qhat gets -m), so pass2 produces
exp-ready TRANSPOSED logits [s, k] directly; exp goes PSUM->bf16 SBUF; the
o-matmul consumes expT as stationary (plus an N=1 ones-matmul for the row
sums, which land per-partition exactly where the reciprocal scale needs
them); only the final [k, c] -> [c, k] flip uses PE transposes (64 per core
vs 1024 in the per-branch design).

Conv weights are host-packed in SBUF layout (one contiguous DMA per output
block) and loaded once per conv (ob-outer loop), not once per s-pair.
conv2 runs in bf16 (weights + inputs); conv1 stays fp32r because its output
feeds the attention logits.
"""

import numpy as np

import concourse.bacc as bacc
import concourse.mybir as mybir
import concourse.tile as tile
from concourse.bass_utils import run_bass_kernel_spmd

B, C, H, W = 4, 512, 64, 64
S = H * W            # 4096
CI = 64              # q/k channels
P = 128
NB = C // P          # 4 channel blocks
RH = 32              # own rows per core
SH = RH * W          # 2048 own spatial positions
NSTH = SH // 512     # 4 local s-tiles
NCHH = SH // P       # 16 local s-chunks
PW = 66              # padded row width
PADR = RH + 2        # padded rows (pad row 0, own rows 1..32, halo row 33)
PAD = PW * PADR      # 2244
NKG = 8              # k groups
KG = SH // NKG       # 256 query positions per group
EPS = 1e-5
GROUPS = [[0, 1], [2, 3], [4, 5], [6, 7]]

F32 = mybir.dt.float32
F32R = mybir.dt.float32r
BF16 = mybir.dt.bfloat16
AF = mybir.ActivationFunctionType
AX = mybir.AxisListType

_CACHE = {}


def build(reps=1, debug=False):
    nc = bacc.Bacc("TRN2", target_bir_lowering=False, debug=False,
                   num_devices=8)

    # ---- I/O ----
    x_d = nc.dram_tensor("xpad", [NB, P, PAD], F32R, kind="ExternalInput")
    w1s_d = nc.dram_tensor("w1s", [NB, P, 36 * P], F32R, kind="ExternalInput")
    w1c_d = nc.dram_tensor("w1c", [NB, P, 36 * P], F32R, kind="ExternalInput")
    w2s_d = nc.dram_tensor("w2s", [NB, P, 36 * P], BF16, kind="ExternalInput")
    w2c_d = nc.dram_tensor("w2c", [NB, P, 36 * P], BF16, kind="ExternalInput")
    b1s_d = nc.dram_tensor("b1s", [NB, P, 1], F32, kind="ExternalInput")
    b1c_d = nc.dram_tensor("b1c", [NB, P, 1], F32, kind="ExternalInput")
    b2s_d = nc.dram_tensor("b2s", [NB, P, 1], F32, kind="ExternalInput")
    b2c_d = nc.dram_tensor("b2c", [NB, P, 1], F32, kind="ExternalInput")
    qw_d = nc.dram_tensor("qw", [NB, P, CI], F32R, kind="ExternalInput")
    kw_d = nc.dram_tensor("kw", [NB, P, CI], F32R, kind="ExternalInput")
    vw_d = nc.dram_tensor("vw", [NB, P, 512], F32R, kind="ExternalInput")
    qb_d = nc.dram_tensor("qb", [CI, 1], F32, kind="ExternalInput")
    kb_d = nc.dram_tensor("kb", [CI, 1], F32, kind="ExternalInput")
    vba_d = nc.dram_tensor("vba", [NB, P, 1], F32, kind="ExternalInput")
    beta_d = nc.dram_tensor("betat", [P, 1], F32, kind="ExternalInput")
    idr_d = nc.dram_tensor("identr", [P, P], F32R, kind="ExternalInput")
    ones_d = nc.dram_tensor("onesrow", [1, S], F32R, kind="ExternalInput")
    onescol_d = nc.dram_tensor("onescol", [P, 1], BF16, kind="ExternalInput")
    zeros_d = nc.dram_tensor("zerospad", [P, PAD], BF16, kind="ExternalInput")
    out_d = nc.dram_tensor("out", [NB, P, SH], F32, kind="ExternalOutput")
    dbg_d = (nc.dram_tensor("dbg", [26, P, S], F32, kind="ExternalOutput")
             if debug else None)

    # ---- internal DRAM ----
    kq_in = nc.dram_tensor("kqin", [CI, SH], F32R, kind="Internal")
    kq_out = nc.dram_tensor("kqout", [2, CI, SH], F32R, kind="Internal")
    vt_in = nc.dram_tensor("vtin", [NCHH, P, 512], BF16, kind="Internal")
    vt_out = nc.dram_tensor("vtout", [2, NCHH, P, 512], BF16, kind="Internal")
    c1t_dd = nc.dram_tensor("c1t", [NCHH, P, 512], F32R, kind="Internal")
    g_in = nc.dram_tensor("gin", [NB, P, 512], F32, kind="Internal")
    g_out = nc.dram_tensor("gout", [NB, P, 512], F32, kind="Internal")
    halo_in = nc.dram_tensor("haloin", [2, NB, P, W], F32, kind="Internal")
    halo_out = nc.dram_tensor("haloout", [2, NB, P, W], F32, kind="Internal")
    mrow_d = nc.dram_tensor("mrow", [NKG, KG], F32R, kind="Internal")

    def pv(t, st, dy=1, dx=1):
        """[128, 8, 64] view of a padded tile covering local s-tile st,
        shifted by conv tap (dy, dx)."""
        v = t.rearrange("p (r w) -> p r w", w=PW)
        r0 = st * 8 + dy
        return v[:, r0:r0 + 8, dx:dx + W]

    def cv(t, ch):
        """[128, 2, 64] centered view covering local s-chunk ch."""
        v = t.rearrange("p (r w) -> p r w", w=PW)
        r0 = ch * 2 + 1
        return v[:, r0:r0 + 2, 1:1 + W]

    def rowv(t, r):
        """[128, 64] centered view of local pad row r."""
        v = t.rearrange("p (r w) -> p r w", w=PW)
        return v[:, r, 1:1 + W]

    def gv(t, g):
        """[128, 4, 64] centered view covering k-group g (4 image rows)."""
        v = t.rearrange("p (r w) -> p r w", w=PW)
        r0 = g * 4 + 1
        return v[:, r0:r0 + 4, 1:1 + W]

    with tile.TileContext(nc) as tc:
        from contextlib import ExitStack

        gctx = ExitStack()
        psA = gctx.enter_context(tc.tile_pool(name="psA", bufs=6,
                                              space="PSUM"))
        psT = gctx.enter_context(tc.tile_pool(name="psT", bufs=2,
                                              space="PSUM"))
        consts = gctx.enter_context(tc.tile_pool(name="consts", bufs=1))
        s1p = gctx.enter_context(tc.tile_pool(name="s1p", bufs=NB))
        c1p = gctx.enter_context(tc.tile_pool(name="c1p", bufs=NB))
        khatp = gctx.enter_context(tc.tile_pool(name="khatp", bufs=1))
        qhatp = gctx.enter_context(tc.tile_pool(name="qhatp", bufs=1))
        b512 = gctx.enter_context(tc.tile_pool(name="b512", bufs=3))
        statp = gctx.enter_context(tc.tile_pool(name="statp", bufs=16))

        # ---- constants ----
        idr_t = consts.tile([P, P], F32R, name="idr")
        nc.sync.dma_start(idr_t[:], idr_d.ap())
        qw_t = [consts.tile([P, CI], F32R, name=f"qw{i}") for i in range(NB)]
        kw_t = [consts.tile([P, CI], F32R, name=f"kw{i}") for i in range(NB)]
        vw_t = [consts.tile([P, 512], F32R, name=f"vw{i}") for i in range(NB)]
        b1s_t = [consts.tile([P, 1], F32, name=f"b1s{i}") for i in range(NB)]
        b1c_t = [consts.tile([P, 1], F32, name=f"b1c{i}") for i in range(NB)]
        b2s_t = [consts.tile([P, 1], F32, name=f"b2s{i}") for i in range(NB)]
        b2c_t = [consts.tile([P, 1], F32, name=f"b2c{i}") for i in range(NB)]
        vba_t = [consts.tile([P, 1], F32, name=f"vba{i}") for i in range(NB)]
        qb_t = consts.tile([CI, 1], F32, name="qb")
        kb_t = consts.tile([CI, 1], F32, name="kb")
        beta_t = consts.tile([P, 1], F32, name="beta")
        onesb_t = consts.tile([P, 1], BF16, name="onesb")
        nc.sync.dma_start(onesb_t[:], onescol_d.ap())
        for i in range(NB):
            nc.sync.dma_start(qw_t[i][:], qw_d[i])
            nc.sync.dma_start(kw_t[i][:], kw_d[i])
            nc.sync.dma_start(vw_t[i][:], vw_d[i])
            nc.sync.dma_start(b1s_t[i][:], b1s_d[i])
            nc.sync.dma_start(b1c_t[i][:], b1c_d[i])
            nc.sync.dma_start(b2s_t[i][:], b2s_d[i])
            nc.sync.dma_start(b2c_t[i][:], b2c_d[i])
            nc.sync.dma_start(vba_t[i][:], vba_d[i])
        nc.sync.dma_start(qb_t[:], qb_d.ap())
        nc.sync.dma_start(kb_t[:], kb_d.ap())
        nc.sync.dma_start(beta_t[:], beta_d.ap())

        # ---- persistent feature-map pads ----
        s1sb = [s1p.tile([P, SH], F32R, tag="s1", name=f"s1sb{i}")
                for i in range(NB)]
        c1sb = [c1p.tile([P, SH], F32R, tag="c1", name=f"c1sb{i}")
                for i in range(NB)]
        khat = khatp.tile([CI + 1, S], F32R, tag="kh", name="khat")
        qhat = qhatp.tile([CI + 1, SH], F32R, tag="qh", name="qhat")


        def conv(wres, src, ob, st, bias, dst, func=AF.Relu, bpool=None):
            """One 3x3 conv s-tile for output block ob: 36 accumulating
            matmuls, relu+bias evict into dst's centered view. With bpool,
            also evicts a contiguous copy (for PE transposes, which cannot
            read multi-free-dim strided views)."""
            ps = psA.tile([P, 512], F32, tag="mm", name="cps")
            for tci in range(36):
                cb, tap = divmod(tci, 9)
                dy, dx = divmod(tap, 3)
                nc.tensor.matmul(ps[:], wres[:, tci * P:(tci + 1) * P],
                                 pv(src[cb], st, dy, dx),
                                 start=(tci == 0), stop=(tci == 35))
            nc.scalar.activation(dst, ps[:], func, bias=bias)
            if bpool is not None:
                sb = bpool.tile([P, 512], F32R, tag="cb", name="csb")
                nc.scalar.activation(sb[:], ps[:], func, bias=bias)
                return sb
            return None

        def body(rep):
            with ExitStack() as convctx:
                xpadp = convctx.enter_context(
                    tc.tile_pool(name="xpadp", bufs=NB))
                xpad = [xpadp.tile([P, PAD], F32R, tag="xp",
                                   name=f"xpad{i}") for i in range(NB)]
                qq = PAD // 4
                for i in range(NB):
                    for q in range(4):
                        eng = nc.sync if q % 2 == 0 else nc.scalar
                        eng.dma_start(xpad[i][:, q * qq:(q + 1) * qq],
                                      x_d[i, :, q * qq:(q + 1) * qq])

                # ---- P1: sa_conv1 (ob-outer), then q/k/vT projections ----
                with ExitStack() as p1:
                    wp = p1.enter_context(tc.tile_pool(name="wp1", bufs=3))
                    vtbp = p1.enter_context(tc.tile_pool(name="vtbp", bufs=5))
                    for ob in range(NB):
                        wres = wp.tile([P, 36 * P], F32R, tag="w", name="wres")
                        nc.sync.dma_start(wres[:], w1s_d[ob])
                        for st in range(NSTH):
                            conv(wres, xpad, ob, st, b1s_t[ob][:],
                                 s1sb[ob][:, st * 512:(st + 1) * 512])
                    for st in range(NSTH):
                        ssl = slice(st * 512, (st + 1) * 512)
                        pq = psA.tile([CI, 512], F32, tag="mm", name="pq")
                        for cb in range(NB):
                            nc.tensor.matmul(pq[:], qw_t[cb][:],
                                             s1sb[cb][:, ssl],
                                             start=(cb == 0),
                                             stop=(cb == NB - 1))
                        nc.scalar.activation(qhat[0:CI, ssl], pq[:],
                                             AF.Identity, bias=qb_t[:])
                        pk = psA.tile([CI, 512], F32, tag="mm", name="pk")
                        for cb in range(NB):
                            nc.tensor.matmul(pk[:], kw_t[cb][:],
                                             s1sb[cb][:, ssl],
                                             start=(cb == 0),
                                             stop=(cb == NB - 1))
                        ksb = b512.tile([CI, 512], F32R, tag="bn", name="ksb")
                        nc.scalar.activation(ksb[:], pk[:], AF.Identity,
                                             bias=kb_t[:])
                        nc.scalar.dma_start(kq_in.ap()[:, ssl], ksb[:])
                        for j in range(4):
                            ch = st * 4 + j
                            pvp = psA.tile([P, 512], F32, tag="mm", name="pv")
                            for cb in range(NB):
                                nc.tensor.matmul(
                                    pvp[:],
                                    s1sb[cb][:, ch * P:(ch + 1) * P],
                                    vw_t[cb][:], start=(cb == 0),
                                    stop=(cb == NB - 1))
                            vtb = vtbp.tile([P, 512], BF16, tag="vtb",
                                            name="vtb")
                            nc.scalar.activation(vtb[:], pvp[:], AF.Identity)
                            nc.scalar.dma_start(vt_in[ch], vtb[:])

                # CC1: pair AllGather of keys and vT
                nc.gpsimd.collective_compute(
                    "AllGather", mybir.AluOpType.bypass,
                    replica_groups=GROUPS,
                    ins=[kq_in.ap()], outs=[kq_out.ap()])
                nc.gpsimd.collective_compute(
                    "AllGather", mybir.AluOpType.bypass,
                    replica_groups=GROUPS,
                    ins=[vt_in.ap()], outs=[vt_out.ap()])
                for r in range(2):
                    nc.sync.dma_start(khat[0:CI, r * SH:(r + 1) * SH],
                                      kq_out[r])
                nc.sync.dma_start(khat[CI:CI + 1, :], ones_d.ap())

                # ---- P2: ca_conv1 + c1T + partial G ----
                with ExitStack() as p2:
                    wp = p2.enter_context(tc.tile_pool(name="wp2", bufs=2))
                    tbp = p2.enter_context(tc.tile_pool(name="tbp", bufs=3))
                    for ob in range(NB):
                        wres = wp.tile([P, 36 * P], F32R, tag="w", name="wres")
                        nc.sync.dma_start(wres[:], w1c_d[ob])
                        for st in range(NSTH):
                            csb = conv(wres, xpad, ob, st, b1c_t[ob][:],
                                       c1sb[ob][:, st * 512:(st + 1) * 512])
                            csb = c1sb[ob][:, st * 512:(st + 1) * 512]
                            tb = tbp.tile([P, 512], F32R, tag="tb", name="tb")
                            for j in range(4):
                                pt = psT.tile([P, P], F32R, tag="tp",
                                              name="pt")
                                nc.tensor.transpose(
                                    pt[:], csb[:, j * P:(j + 1) * P],
                                    idr_t[:])
                                nc.scalar.activation(tb[:, j * P:(j + 1) * P],
                                                     pt[:], AF.Identity)
                            nc.scalar.dma_start(
                                c1t_dd.ap()[st * 4:st * 4 + 4, :,
                                            ob * P:(ob + 1) * P]
                                .rearrange("j p c -> p j c"),
                                tb[:].rearrange("p (j c) -> p j c", c=P))
                    pg = [psA.tile([P, 512], F32, tag="mm", name=f"pg{cb}")
                          for cb in range(NB)]
                    with tc.tile_pool(name="c1tin", bufs=3) as cp:
                        for ch in range(NCHH):
                            ct = cp.tile([P, 512], F32R, tag="ct", name="ct")
                            nc.sync.dma_start(ct[:], c1t_dd[ch])
                            for cb in range(NB):
                                nc.tensor.matmul(
                                    pg[cb][:],
                                    ct[:, cb * P:(cb + 1) * P], ct[:],
                                    start=(ch == 0), stop=(ch == NCHH - 1))
                    for cb in range(NB):
                        gsb = b512.tile([P, 512], F32, tag="bn", name="gsb")
                        nc.scalar.activation(gsb[:], pg[cb][:], AF.Identity)
                        nc.scalar.dma_start(g_in[cb], gsb[:])

                # CC2: pair AllReduce of G
                nc.gpsimd.collective_compute(
                    "AllReduce", mybir.AluOpType.add,
                    replica_groups=GROUPS,
                    ins=[g_in.ap()], outs=[g_out.ap()])

            if debug:
                for i in range(NB):
                    nc.gpsimd.dma_start(dbg_d[i, :, :SH], s1sb[i][:])
                    nc.gpsimd.dma_start(dbg_d[4 + i, :, :SH], c1sb[i][:])
                nc.gpsimd.dma_start(dbg_d[8, 0:CI + 1, :], khat[:])

            # xpad freed; open result pads + attention pools
            with ExitStack() as rctx2:
                srp = rctx2.enter_context(tc.tile_pool(name="srp", bufs=NB))
                crp = rctx2.enter_context(tc.tile_pool(name="crp", bufs=NB))
                sres = [srp.tile([P, PAD], BF16, tag="sr", name=f"sres{i}")
                        for i in range(NB)]
                cres = [crp.tile([P, PAD], BF16, tag="cr", name=f"cres{i}")
                        for i in range(NB)]
                for i in range(NB):
                    nc.sync.dma_start(sres[i][:], zeros_d.ap())
                    nc.scalar.dma_start(cres[i][:], zeros_d.ap())

                # ---- P3: spatial attention, pipelined over 8 k-groups ----
                with ExitStack() as p3:
                    vtfp = p3.enter_context(tc.tile_pool(name="vtfp",
                                                         bufs=1))
                    vtfull = vtfp.tile([P, 32 * 512], BF16, tag="vtf",
                                       name="vtfull")
                    for r in range(2):
                        nc.sync.dma_start(
                            vtfull[:, r * 16 * 512:(r + 1) * 16 * 512]
                            .rearrange("p (j n) -> p j n", n=512),
                            vt_out[r].rearrange("j p n -> p j n"))
                    expTp = p3.enter_context(tc.tile_pool(name="expTp",
                                                          bufs=2))
                    eTs, rsts = {}, {}

                    def issue_L(g):
                        nm = statp.tile([P, 2], F32R, tag="st", name="nm")
                        for kci in range(2):
                            kc = g * 2 + kci
                            pmax = statp.tile([P, 8], F32, tag="st",
                                              name="pmax")
                            for st8 in range(8):
                                pl = psA.tile([P, 512], F32, tag="mm",
                                              name="pl")
                                nc.tensor.matmul(
                                    pl[:], qhat[0:CI, kc * P:(kc + 1) * P],
                                    khat[0:CI, st8 * 512:(st8 + 1) * 512],
                                    start=True, stop=True)
                                nc.vector.reduce_max(pmax[:, st8:st8 + 1],
                                                     pl[:], axis=AX.X)
                            nc.vector.reduce_max(nm[:, kci:kci + 1], pmax[:],
                                                 axis=AX.X, negate=True)
                        nc.scalar.dma_start(
                            mrow_d.ap()[g].rearrange("(j p) -> p j", p=P),
                            nm[:])
                        nc.sync.dma_start(
                            qhat[CI:CI + 1, g * KG:(g + 1) * KG],
                            mrow_d.ap()[g:g + 1, :])

                    def issue_M(g):
                        eT = expTp.tile([P, 32 * KG], BF16, tag="eT",
                                        name="eT")
                        for sc in range(32):
                            lt = psA.tile([P, KG], F32, tag="mm", name="lt")
                            nc.tensor.matmul(
                                lt[:], khat[0:CI + 1, sc * P:(sc + 1) * P],
                                qhat[0:CI + 1, g * KG:(g + 1) * KG],
                                start=True, stop=True)
                            nc.scalar.activation(eT[:, sc * KG:(sc + 1) * KG],
                                                 lt[:], AF.Exp)
                        eTs[g] = eT

                    def issue_O(g):
                        eT = eTs.pop(g)
                        if debug and g == 0:
                            nc.gpsimd.dma_start(dbg_d[9, 0:CI + 1, :SH],
                                                qhat[:])
                            nc.gpsimd.dma_start(
                                dbg_d[10, :, :],
                                eT[:, :S])
                            nc.gpsimd.dma_start(
                                dbg_d[11, :, :],
                                eT[:, S:2 * S])
                        po = [psA.tile([P, 512], F32, tag="mm",
                                       name=f"po{i}") for i in range(2)]
                        # start=True clears the WHOLE PSUM bank, so each
                        # rowsum accumulator needs its own bank
                        prs = [psA.tile([P, 1], F32, tag="mm",
                                        name=f"prs{i}") for i in range(2)]
                        for sc in range(32):
                            vt = vtfull[:, sc * 512:(sc + 1) * 512]
                            for kci in range(2):
                                esl = eT[:, sc * KG + kci * P:
                                         sc * KG + (kci + 1) * P]
                                nc.tensor.matmul(po[kci][:], esl, vt,
                                                 start=(sc == 0),
                                                 stop=(sc == 31))
                                nc.tensor.matmul(
                                    prs[kci][:], esl,
                                    onesb_t[:], start=(sc == 0),
                                    stop=(sc == 31))
                        rsts[g] = (po, prs)

                    def issue_R(g):
                        po, prs = rsts.pop(g)
                        rs = statp.tile([P, 2], F32, tag="st", name="rs")
                        for kci in range(2):
                            nc.vector.reciprocal(rs[:, kci:kci + 1],
                                                 prs[kci][:])
                        if debug:
                            prsb = statp.tile([P, 2], F32, tag="st",
                                              name="prsb")
                            for kci in range(2):
                                nc.scalar.activation(prsb[:, kci:kci + 1],
                                                     prs[kci][:], AF.Identity)
                            nc.gpsimd.dma_start(
                                dbg_d[24, :, 4 * g:4 * g + 2], prsb[:])
                            nc.gpsimd.dma_start(
                                dbg_d[24, :, 4 * g + 2:4 * g + 4], rs[:])
                            pob = b512.tile([P, 512], F32, tag="bn",
                                            name="pob")
                            nc.scalar.activation(pob[:], po[0][:],
                                                 AF.Identity)
                            nc.gpsimd.dma_start(
                                dbg_d[25, :, 512 * (g % 8):512 * (g % 8 + 1)],
                                pob[:])
                        posb = []
                        for kci in range(2):
                            t = b512.tile([P, 512], F32R, tag="bn",
                                          name="posb")
                            nc.scalar.activation(t[:], po[kci][:],
                                                 AF.Identity,
                                                 scale=rs[:, kci:kci + 1])
                            posb.append(t)
                        for cb in range(NB):
                            ptp = psT.tile([P, KG], F32R, tag="tp",
                                           name="ptp")
                            for kci in range(2):
                                nc.tensor.transpose(
                                    ptp[:, kci * P:(kci + 1) * P],
                                    posb[kci][:, cb * P:(cb + 1) * P],
                                    idr_t[:])
                            ob_sb = b512.tile([P, KG], F32, tag="bn",
                                              name="obsb")
                            nc.scalar.activation(ob_sb[:], ptp[:],
                                                 AF.Identity,
                                                 bias=vba_t[cb][:])
                            nc.vector.tensor_add(
                                gv(sres[cb], g),
                                ob_sb[:].rearrange("p (r w) -> p r w", w=W),
                                s1sb[cb][:, g * KG:(g + 1) * KG]
                                .rearrange("p (r w) -> p r w", w=W))

                    issue_L(0)
                    issue_L(1)
                    issue_M(0)
                    for g in range(NKG):
                        if g + 2 < NKG:
                            issue_L(g + 2)
                        if g + 1 < NKG:
                            issue_M(g + 1)
                        issue_O(g)
                        issue_R(g)

                # ---- P4: channel middle ----
                with tc.tile_pool(name="cattp", bufs=NB) as cattp:
                    catt = []
                    for cb in range(NB):
                        gsb = b512.tile([P, 512], F32, tag="bn", name="gsb2")
                        nc.sync.dma_start(gsb[:], g_out[cb])
                        negmax = statp.tile([P, 1], F32, tag="st",
                                            name="negmax")
                        nc.vector.reduce_max(negmax[:], gsb[:], axis=AX.X,
                                             negate=True)
                        ct = cattp.tile([P, 512], F32R, tag="ct",
                                        name=f"catt{cb}")
                        rowsum = statp.tile([P, 1], F32, tag="st",
                                            name="rowsum")
                        nc.scalar.activation(ct[:], gsb[:], AF.Exp,
                                             bias=negmax[:],
                                             accum_out=rowsum[:])
                        recip = statp.tile([P, 1], F32, tag="st",
                                           name="recip")
                        nc.vector.reciprocal(recip[:], rowsum[:])
                        nc.vector.tensor_mul(recip[:], recip[:], beta_t[:])
                        nc.scalar.activation(ct[:], ct[:], AF.Identity,
                                             scale=recip[:])
                        catt.append(ct)
                    for st in range(NSTH):
                        pc4 = [psA.tile([P, 512], F32, tag="mm",
                                        name=f"pc{kb}") for kb in range(NB)]
                        for kb in range(NB):
                            for cb in range(NB):
                                nc.tensor.matmul(
                                    pc4[kb][:],
                                    catt[cb][:, kb * P:(kb + 1) * P],
                                    c1sb[cb][:, st * 512:(st + 1) * 512],
                                    start=(cb == 0),
                                    stop=(cb == NB - 1))
                        for kb in range(NB):
                            nc.vector.tensor_add(
                                pv(cres[kb], st), pc4[kb][:],
                                c1sb[kb][:, st * 512:(st + 1) * 512]
                                .rearrange("p (r w) -> p r w", w=W))

                # ---- CC3: halo rows (partner = sum - own) ----
                with tc.tile_pool(name="halop", bufs=1) as hp:
                    hsb = hp.tile([P, 2 * NB * W], F32, name="hsb")
                    for t, buf in ((0, sres), (1, cres)):
                        for b in range(NB):
                            nc.vector.tensor_copy(
                                hsb[:, (t * NB + b) * W:(t * NB + b + 1) * W],
                                rowv(buf[b], RH))
                    for t in range(2):
                        for b in range(NB):
                            nc.scalar.dma_start(
                                halo_in[t, b],
                                hsb[:, (t * NB + b) * W:(t * NB + b + 1) * W])
                    nc.gpsimd.collective_compute(
                        "AllReduce", mybir.AluOpType.add,
                        replica_groups=GROUPS,
                        ins=[halo_in.ap()], outs=[halo_out.ap()])
                    hob = hp.tile([P, 2 * NB * W], F32, name="hob")
                    for t in range(2):
                        for b in range(NB):
                            nc.sync.dma_start(
                                hob[:, (t * NB + b) * W:(t * NB + b + 1) * W],
                                halo_out[t, b])
                    hneg = hp.tile([P, 2 * NB * W], F32, name="hneg")
                    nc.vector.tensor_scalar_mul(hneg[:], hsb[:], -1.0)
                    nc.vector.tensor_add(hob[:], hob[:], hneg[:])
                    for t, buf in ((0, sres), (1, cres)):
                        for b in range(NB):
                            nc.vector.tensor_copy(
                                rowv(buf[b], RH + 1),
                                hob[:, (t * NB + b) * W:(t * NB + b + 1) * W])

                if debug:
                    for i in range(NB):
                        nc.gpsimd.dma_start(dbg_d[12 + i, :, :PAD],
                                            sres[i][:])
                        nc.gpsimd.dma_start(dbg_d[16 + i, :, :PAD],
                                            cres[i][:])
                        gdb = b512.tile([P, 512], F32, tag="bn", name="gdb")
                        nc.sync.dma_start(gdb[:], g_out[i])
                        nc.gpsimd.dma_start(dbg_d[20 + i, :, :512], gdb[:])

                # ---- P5: conv2s (bf16), summed on the fly ----
                with ExitStack() as p5:
                    wp = p5.enter_context(tc.tile_pool(name="wp5", bufs=4))
                    osbp = p5.enter_context(tc.tile_pool(name="osbp", bufs=4))
                    for ob in range(NB):
                        ws = wp.tile([P, 36 * P], BF16, tag="w", name="w2s")
                        nc.sync.dma_start(ws[:], w2s_d[ob])
                        wc = wp.tile([P, 36 * P], BF16, tag="w", name="w2c")
                        nc.sync.dma_start(wc[:], w2c_d[ob])
                        for st in range(NSTH):
                            pss = psA.tile([P, 512], F32, tag="mm",
                                           name="pss")
                            psc = psA.tile([P, 512], F32, tag="mm",
                                           name="psc")
                            for tci in range(36):
                                cb, tap = divmod(tci, 9)
                                dy, dx = divmod(tap, 3)
                                nc.tensor.matmul(
                                    pss[:], ws[:, tci * P:(tci + 1) * P],
                                    pv(sres[cb], st, dy, dx),
                                    start=(tci == 0), stop=(tci == 35))
                            for tci in range(36):
                                cb, tap = divmod(tci, 9)
                                dy, dx = divmod(tap, 3)
                                nc.tensor.matmul(
                                    psc[:], wc[:, tci * P:(tci + 1) * P],
                                    pv(cres[cb], st, dy, dx),
                                    start=(tci == 0), stop=(tci == 35))
                            osb = osbp.tile([P, 512], F32, tag="o",
                                            name="osb")
                            osc = osbp.tile([P, 512], F32, tag="o",
                                            name="osc")
                            nc.scalar.activation(osb[:], pss[:], AF.Relu,
                                                 bias=b2s_t[ob][:])
                            nc.scalar.activation(osc[:], psc[:], AF.Relu,
                                                 bias=b2c_t[ob][:])
                            nc.vector.tensor_add(osb[:], osb[:], osc[:])
                            nc.sync.dma_start(
                                out_d[ob, :, st * 512:(st + 1) * 512],
                                osb[:])

        for rep in range(reps):
            body(rep)

        gctx.close()

    nc.compile()
    return nc


def _fold_conv(w, g, b, m, v, flip, bf16=False):
    scale = g / np.sqrt(v + EPS)
    wf = (np.asarray(w, np.float32) * scale[:, None, None, None])
    bf = (np.asarray(b, np.float32) - np.asarray(m, np.float32) * scale)
    if flip:
        wf = wf[:, :, ::-1, :]          # mirror dy
    # [O, CIn, 3, 3] -> [ob, ci, ((cb tap) o)]
    wt = wf.transpose(2, 3, 1, 0).reshape(9, NB, P, NB, P).transpose(
        3, 1, 0, 2, 4).reshape(NB, 36, P, P).transpose(0, 2, 1, 3).reshape(
        NB, P, 36 * P)
    if bf16:
        import ml_dtypes
        wt = wt.astype(ml_dtypes.bfloat16)
    else:
        wt = wt.astype(np.float32)
    return np.ascontiguousarray(wt), bf.astype(np.float32).reshape(NB, P, 1)


def _pad_half(x, h):
    """x [C, 64, 64] -> padded own-half [NB, P, PAD] for parity h.
    h=1 is vertically mirrored so the halo row is at local row 33 on both."""
    xr = x.reshape(NB, P, H, W)
    if h == 1:
        xr = xr[:, :, ::-1, :]
    xp = np.zeros((NB, P, PADR, PW), np.float32)
    xp[:, :, 1:PADR, 1:1 + W] = xr[:, :, 0:RH + 1]
    return np.ascontiguousarray(xp.reshape(NB, P, PAD))


def prep_inputs(inputs):
    x = np.asarray(inputs["x"], np.float32)
    alpha = float(np.asarray(inputs["alpha"]).reshape(-1)[0])
    beta = float(np.asarray(inputs["beta"]).reshape(-1)[0])

    per_parity = []
    for h in range(2):
        w1s, b1s = _fold_conv(inputs["sa_w1"], inputs["sa_g1"],
                              inputs["sa_b1"], inputs["sa_m1"],
                              inputs["sa_v1"], flip=(h == 1))
        w2s, b2s = _fold_conv(inputs["sa_w2"], inputs["sa_g2"],
                              inputs["sa_b2"], inputs["sa_m2"],
                              inputs["sa_v2"], flip=(h == 1), bf16=True)
        w1c, b1c = _fold_conv(inputs["ca_w1"], inputs["ca_g1"],
                              inputs["ca_b1"], inputs["ca_m1"],
                              inputs["ca_v1"], flip=(h == 1))
        w2c, b2c = _fold_conv(inputs["ca_w2"], inputs["ca_g2"],
                              inputs["ca_b2"], inputs["ca_m2"],
                              inputs["ca_v2"], flip=(h == 1), bf16=True)
        per_parity.append((w1s, b1s, w2s, b2s, w1c, b1c, w2c, b2c))

    qw = np.ascontiguousarray(
        np.asarray(inputs["q_w"], np.float32).T.reshape(NB, P, CI))
    kw = np.ascontiguousarray(
        np.asarray(inputs["k_w"], np.float32).T.reshape(NB, P, CI))
    vw = np.ascontiguousarray(
        (alpha * np.asarray(inputs["v_w"], np.float32)).T.reshape(NB, P, 512))
    qb = np.asarray(inputs["q_b"], np.float32).reshape(CI, 1)
    kb = np.asarray(inputs["k_b"], np.float32).reshape(CI, 1)
    vba = (alpha * np.asarray(inputs["v_b"], np.float32)).reshape(NB, P, 1)
    betat = np.full((P, 1), beta, np.float32)
    identr = np.eye(P, dtype=np.float32)
    onesrow = np.ones((1, S), np.float32)
    import ml_dtypes
    onescol = np.ones((P, 1), ml_dtypes.bfloat16)
    zerospad = np.zeros((P, PAD), ml_dtypes.bfloat16)

    maps = []
    for core in range(8):
        b, h = core // 2, core % 2
        w1s, b1s, w2s, b2s, w1c, b1c, w2c, b2c = per_parity[h]
        m = dict(xpad=_pad_half(x[b], h),
                 w1s=w1s, b1s=b1s, w2s=w2s, b2s=b2s,
                 w1c=w1c, b1c=b1c, w2c=w2c, b2c=b2c,
                 qw=qw, kw=kw, vw=vw, qb=qb, kb=kb, vba=vba, betat=betat,
                 identr=identr, onesrow=onesrow, onescol=onescol,
                 zerospad=zerospad)
        maps.append(m)
    return maps


def kernel(**inputs):
    if "nc" not in _CACHE:
        _CACHE["nc"] = build()
    nc = _CACHE["nc"]
    maps = prep_inputs(inputs)
    res = run_bass_kernel_spmd(nc, maps, core_ids=list(range(8)))
    out = np.zeros((B, C, H, W), np.float32)
    for b in range(B):
        top = res.results[2 * b]["out"].reshape(C, RH, W)
        bot = res.results[2 * b + 1]["out"].reshape(C, RH, W)[:, ::-1, :]
        out[b, :, :RH] = top
        out[b, :, RH:] = bot
    return out
